# revision 1
# baseline (speedup 1.0000x reference)
"""Trainium2 Bass kernel for nn_MAD_GCN (retrieval_knn).

Strategy: shard the B=512 edges across 8 NeuronCores (64 edges each, no
collectives). Per core, per (side, head-pair) column-tile of 128 columns
(64 edges x 2 heads, K-packed into the 128-contraction):

  phase 1 (PE):   s[c,n] = 2*q_c . pos_n - |pos_n|^2   (monotone in -dist^2)
                  via two accumulating matmuls per 512-chunk of N.
  phase 2 (ACT+DVE): evict 2048-wide macro-chunks PSUM->SBUF, per-chunk MAX8
                  -> 64 candidates/column; merge (max8/match_replace/max8)
                  -> top-9 values; drop rank-1 (the query itself); one
                  full-width MAX_INDEX recovers the 8 neighbor indices.
  phase 3 (DMA+DVE): dma_gather the selected pos vectors and adj windows
                  (256B granularity), recompute diff/contrib/distance exactly
                  as the reference does, softmin over 2K+8 sentinel samples,
                  sigmoid, per-edge reduction via a small matmul.

Host-side work is limited to sharding/layout: transposes, per-core slices of
adj rows/columns for its 64 edges, and gathers by the host-known `edges`.
"""

import numpy as np

import concourse.bass as bass
import concourse.bacc as bacc
import concourse.mybir as mybir
import concourse.tile as tile
from concourse.bass_utils import run_bass_kernel_spmd

F32 = mybir.dt.float32
F32R = mybir.dt.float32r
BF16 = mybir.dt.bfloat16
I32 = mybir.dt.int32
I16 = mybir.dt.int16
U32 = mybir.dt.uint32
ALU = mybir.AluOpType
ACTF = mybir.ActivationFunctionType
AX = mybir.AxisListType

N_NODES = 16384
H = 4
D = 64
B = 512
K = 8
NUM_SENT = 8
SENT_DIST = 1.0
N_CORES = 8
BLOC = B // N_CORES  # 64 edges per core

# perf flags (compile-time)
USE_F32R = False      # fp32r matmul: 4x faster PE, slightly different numerics
PN2_BF16 = False      # 3-way bf16 split of the pn2 rank-2 update (1cyc/row)

MC = 2048             # macro-chunk width (4 PSUM banks)


def _mm(ap, use_f32r):
    return ap.bitcast(F32R) if use_f32r else ap


def _dbg_out(nc, pred, ap, width):
    # debug early-out: reduce a (128,w) f32 AP into pred (64,1)
    import concourse.mybir as _mb
    dbg = _DBG_POOL[0].tile([128, 1], F32, tag="dbg", name="dbg")
    nc.vector.tensor_reduce(dbg, ap, axis=AX.X, op=ALU.add)
    nc.sync.dma_start(pred[:, :], dbg[0:64, :])


_DBG_POOL = [None]


def emit(nc, tc, n_nodes, use_f32r=USE_F32R, pn2_bf16=PN2_BF16, stage=99):
    n_mc = n_nodes // MC
    wpr = n_nodes // 64  # adj windows per row
    ncand = 8 * n_mc
    pn2_dt = BF16 if pn2_bf16 else F32
    pn2_rows = 6 if pn2_bf16 else 2

    # ---- DRAM I/O ----
    dr = {}
    for hp in range(2):
        dr[f"rhs1_{hp}"] = nc.dram_tensor(f"rhs1_{hp}", [128, n_nodes], F32,
                                          kind="ExternalInput")
        dr[f"rhs2_{hp}"] = nc.dram_tensor(f"rhs2_{hp}", [pn2_rows, n_nodes],
                                          pn2_dt, kind="ExternalInput")
        dr[f"posg_{hp}"] = nc.dram_tensor(f"posg_{hp}", [2 * n_nodes, D], F32,
                                          kind="ExternalInput")
    for s in range(2):
        for hp in range(2):
            dr[f"lhsT_{s}{hp}"] = nc.dram_tensor(f"lhsT_{s}{hp}", [128, 128],
                                                 F32, kind="ExternalInput")
            dr[f"q_{s}{hp}"] = nc.dram_tensor(f"q_{s}{hp}", [128, D], F32,
                                              kind="ExternalInput")
            dr[f"g_{s}{hp}"] = nc.dram_tensor(f"g_{s}{hp}", [128, D], F32,
                                              kind="ExternalInput")
    dr["lhsT2"] = nc.dram_tensor("lhsT2", [pn2_rows, 128], pn2_dt,
                                 kind="ExternalInput")
    dr["adjcol"] = nc.dram_tensor("adjcol", [n_nodes, 64], F32,
                                  kind="ExternalInput")
    dr["adjrow"] = nc.dram_tensor("adjrow", [n_nodes, 64], F32,
                                  kind="ExternalInput")
    dr["oneb"] = nc.dram_tensor("oneb", [128, 64], F32, kind="ExternalInput")
    dr["lw"] = nc.dram_tensor("lw", [128, 1], F32, kind="ExternalInput")
    pred = nc.dram_tensor("pred", [BLOC, 1], F32, kind="ExternalOutput")

    # ---- pools ----
    from contextlib import ExitStack
    ctx = getattr(tc, "_emit_ctx")
    pc = ctx.enter_context(tc.tile_pool(name="const", bufs=1))
    pr = ctx.enter_context(tc.tile_pool(name="rhs", bufs=1))
    pw = ctx.enter_context(tc.tile_pool(name="work", bufs=2))
    pp = ctx.enter_context(tc.tile_pool(name="psum", bufs=2, space="PSUM"))
    pf = ctx.enter_context(tc.tile_pool(name="ph3", bufs=1))
    _DBG_POOL[0] = pf

    tiles = [(s, hp) for hp in range(2) for s in range(2)]

    # ---- persistent consts ----
    lhsT_sb = {}
    q_sb = {}
    g_sb = {}
    for (s, hp) in tiles:
        lhsT_sb[(s, hp)] = pc.tile([128, 128], F32, tag=f"lhsT{s}{hp}", name=f"lhsT{s}{hp}")
        nc.sync.dma_start(lhsT_sb[(s, hp)], dr[f"lhsT_{s}{hp}"][:, :])
        q_sb[(s, hp)] = pc.tile([128, D], F32, tag=f"q{s}{hp}", name=f"q{s}{hp}")
        nc.sync.dma_start(q_sb[(s, hp)], dr[f"q_{s}{hp}"][:, :])
        g_sb[(s, hp)] = pc.tile([128, D], F32, tag=f"g{s}{hp}", name=f"g{s}{hp}")
        nc.sync.dma_start(g_sb[(s, hp)], dr[f"g_{s}{hp}"][:, :])
    lhsT2_sb = pc.tile([pn2_rows, 128], pn2_dt, tag="lhsT2", name="lhsT2")
    nc.sync.dma_start(lhsT2_sb, dr["lhsT2"][:, :])
    # rhs2 is reloaded per head-pair inside the tile loop (SBUF budget)
    oneb_sb = pc.tile([128, 64], F32, tag="oneb", name="oneb")
    nc.sync.dma_start(oneb_sb, dr["oneb"][:, :])
    lw_sb = pc.tile([128, 1], F32, tag="lw", name="lw")
    nc.sync.dma_start(lw_sb, dr["lw"][:, :])

    iota64 = pc.tile([128, 64], I32, tag="iota64", name="iota64")
    nc.gpsimd.iota(iota64, pattern=[[1, 64]], base=0, channel_multiplier=0)
    hiN = pc.tile([128, 1], I32, tag="hiN", name="hiN")
    nc.vector.memset(hiN[0:64, :], 0)
    nc.vector.memset(hiN[64:128, :], n_nodes)
    iop = pc.tile([128, 1], I32, tag="iop", name="iop")  # wpr * partition_index
    nc.gpsimd.iota(iop, pattern=[[1, 1]], base=0, channel_multiplier=wpr)
    bw = pc.tile([128, 1], I32, tag="bw", name="bw")    # wpr * (c % 64)
    # bw = wpr*p - wpr*64*hi ; hiN holds n_nodes = wpr*64 exactly
    nc.vector.tensor_sub(bw, iop, hiN)
    iotaC_i = pc.tile([128, ncand], I32, tag="iotaC_i", name="iotaC_i")
    nc.gpsimd.iota(iotaC_i, pattern=[[1, ncand]], base=0, channel_multiplier=0)
    iotaC = pc.tile([128, ncand], F32, tag="iotaC", name="iotaC")
    nc.vector.tensor_copy(iotaC, iotaC_i)

    # ---- phases 1+2 per tile ----
    n32 = {}
    cur_hp = -1
    rhs_sb = None
    rhs2_sb = None
    for (s, hp) in tiles:
        if hp != cur_hp:
            rhs_sb = pr.tile([128, n_nodes], F32, tag="rhs1", name="rhs1")
            nc.sync.dma_start(rhs_sb, dr[f"rhs1_{hp}"][:, :])
            rhs2_sb = pr.tile([pn2_rows, n_nodes], pn2_dt, tag="rhs2",
                              name="rhs2")
            nc.sync.dma_start(rhs2_sb, dr[f"rhs2_{hp}"][:, :])
            cur_hp = hp
        cand = pw.tile([128, ncand], F32, tag="cand", name="cand")
        candi = pw.tile([128, ncand], F32, tag="candi", name="candi")
        for mc in range(n_mc):
            ps = pp.tile([128, MC], F32, tag="ps", name="ps")
            for j in range(4):
                c0 = j * 512
                n0 = mc * MC + c0
                nc.tensor.matmul(ps[:, c0:c0 + 512],
                                 _mm(lhsT_sb[(s, hp)][:, :], use_f32r),
                                 _mm(rhs_sb[:, n0:n0 + 512], use_f32r),
                                 start=True, stop=False)
                nc.tensor.matmul(ps[:, c0:c0 + 512],
                                 lhsT2_sb[:, :],
                                 rhs2_sb[:, n0:n0 + 512],
                                 start=False, stop=True)
            sc = pw.tile([128, MC], F32, tag="sc", name="sc", bufs=2)
            nc.scalar.copy(sc, ps[:, :])
            nc.vector.max(cand[:, mc * 8:(mc + 1) * 8], sc)
            li = pw.tile([128, 8], U32, tag="li", name="li", bufs=2)
            nc.vector.max_index(li, cand[:, mc * 8:(mc + 1) * 8], sc)
            lif = pw.tile([128, 8], F32, tag="lif", name="lif", bufs=2)
            nc.vector.tensor_copy(lif, li)
            nc.vector.tensor_scalar(candi[:, mc * 8:(mc + 1) * 8], lif,
                                    float(mc * MC), None, op0=ALU.add)
        if stage <= 1:
            _dbg_out(nc, pred, cand[:, :], ncand)
            return pred
        m1 = pw.tile([128, 8], F32, tag="m1", name="m1")
        nc.vector.max(m1, cand)
        repl = pw.tile([128, ncand], F32, tag="repl", name="repl")
        nc.vector.match_replace(out=repl, in_to_replace=m1, in_values=cand,
                                imm_value=-3.0e38)
        m2 = pw.tile([128, 8], F32, tag="m2", name="m2")
        nc.vector.max(m2, repl)
        selv = pw.tile([128, 8], F32, tag="selv", name="selv")
        nc.vector.tensor_copy(selv[:, 0:7], m1[:, 1:8])
        nc.vector.tensor_copy(selv[:, 7:8], m2[:, 0:1])
        jp = pw.tile([128, 8], U32, tag="jp", name="jp")
        nc.vector.max_index(jp, selv, cand)
        jpf = pw.tile([128, 8], F32, tag="jpf", name="jpf")
        nc.vector.tensor_copy(jpf, jp)
        eq = pw.tile([128, 8, ncand], F32, tag="eq", name="eq")
        nc.vector.tensor_tensor(
            eq,
            iotaC[:, :].rearrange("p (one j) -> p one j", one=1)
                       .to_broadcast([128, 8, ncand]),
            jpf[:, :].rearrange("p (k one) -> p k one", one=1)
                     .to_broadcast([128, 8, ncand]),
            op=ALU.is_equal)
        nc.vector.tensor_mul(
            eq, eq,
            candi[:, :].rearrange("p (one j) -> p one j", one=1)
                       .to_broadcast([128, 8, ncand]))
        nf = pw.tile([128, 8], F32, tag="nf", name="nf")
        nc.vector.reduce_sum(nf, eq[:, :, :], axis=AX.X)
        v = pc.tile([128, 8], I32, tag=f"n32{s}{hp}", name=f"n32{s}{hp}")
        nc.vector.tensor_copy(v, nf)
        n32[(s, hp)] = v

    # ---- phase 3: index math + gathers ----

    if stage <= 2:
        _dbg_out(nc, pred, nf[:, :], 8)
        return pred
    # int32 row-index tiles, one row gathered per partition per call
    pidx = {}
    for hp in range(2):
        pidx[hp] = pc.tile([128, 16], I32, tag=f"pidx{hp}", name=f"pidx{hp}")
        for s in range(2):
            nc.vector.tensor_add(pidx[hp][:, s * 8:(s + 1) * 8], n32[(s, hp)],
                                 hiN[:, :].to_broadcast([128, 8]))
    cidx = pc.tile([128, 16], I32, tag="cidx", name="cidx")
    for hp in range(2):
        nc.vector.tensor_copy(cidx[:, hp * 8:(hp + 1) * 8], n32[(0, hp)])
    ridx = pc.tile([128, 16], I32, tag="ridx", name="ridx")
    offs = {}
    for hp in range(2):
        sh = pw.tile([128, 8], I32, tag="sh", name="sh")
        nc.vector.tensor_scalar(sh, n32[(1, hp)], 6, None,
                                op0=ALU.logical_shift_right)
        nc.vector.tensor_add(ridx[:, hp * 8:(hp + 1) * 8], sh,
                             bw[:, :].to_broadcast([128, 8]))
        of = pc.tile([128, 8], I32, tag=f"offs{hp}", name=f"offs{hp}")
        nc.vector.tensor_scalar(of, n32[(1, hp)], 63, None,
                                op0=ALU.bitwise_and)
        offs[hp] = of

    def row_gather(dst2d, table_ap, idx, nslots):
        for k in range(nslots):
            nc.gpsimd.indirect_dma_start(
                out=dst2d[:, k * D:(k + 1) * D],
                out_offset=None,
                in_=table_ap,
                in_offset=bass.IndirectOffsetOnAxis(ap=idx[:, k:k + 1],
                                                    axis=0))

    pg = {}
    for hp in range(2):
        g2 = pf.tile([128, 16 * D], F32, tag=f"pg{hp}", name=f"pg{hp}")
        row_gather(g2, dr[f"posg_{hp}"][:, :], pidx[hp], 16)
        pg[hp] = g2[:, :].rearrange("p (k d) -> p k d", k=16)
    ac2 = pf.tile([128, 16 * D], F32, tag="ac", name="ac")
    row_gather(ac2, dr["adjcol"][:, :], cidx, 16)
    ac = ac2[:, :].rearrange("p (k d) -> p k d", k=16)
    ar2 = pf.tile([128, 16 * D], F32, tag="ar", name="ar")
    row_gather(ar2, dr["adjrow"][:, :], ridx, 16)
    ar = ar2[:, :].rearrange("p (k d) -> p k d", k=16)

    if stage <= 3:
        _dbg_out(nc, pred, ac2[:, 0:64], 64)
        return pred
    # ---- phase 3: per-tile neighbor math, per-hp softmin ----
    R = pf.tile([128, 2], F32, tag="R", name="R")
    for hp in range(2):
        dfull = pf.tile([128, 2 * K + NUM_SENT], F32, tag=f"dfull{hp}", name=f"dfull{hp}")
        lfull = pf.tile([128, 2 * K + NUM_SENT], F32, tag=f"lfull{hp}", name=f"lfull{hp}")
        nc.vector.memset(dfull[:, 16:24], SENT_DIST)
        nc.vector.memset(lfull[:, 16:24], 0.0)
        for s in range(2):
            P = pg[hp][:, s * 8:(s + 1) * 8, :]
            qb = q_sb[(s, hp)][:, :].rearrange(
                "p (one d) -> p one d", one=1).to_broadcast([128, 8, D])
            gb = g_sb[(s, hp)][:, :].rearrange(
                "p (one d) -> p one d", one=1).to_broadcast([128, 8, D])
            diff = pf.tile([128, 8, D], F32, tag="diff", name="diff")
            nc.vector.tensor_sub(diff, qb, P)
            prod = pf.tile([128, 8, D], F32, tag="prod", name="prod")
            nc.vector.tensor_mul(prod, diff, gb)
            contrib = pf.tile([128, 8], F32, tag="contrib", name="contrib")
            nc.vector.reduce_sum(contrib, prod[:, :, :], axis=AX.X)
            nc.vector.tensor_mul(prod, diff, diff)
            d2 = pf.tile([128, 8], F32, tag="d2", name="d2")
            nc.vector.reduce_sum(d2, prod[:, :, :], axis=AX.X)
            nc.scalar.sqrt(dfull[:, s * 8:(s + 1) * 8], d2)
            # adj values
            asel = pf.tile([128, 8], F32, tag="asel", name="asel")
            if s == 0:
                awin = ac[:, hp * 8:(hp + 1) * 8, :]
                ob = oneb_sb[:, :].rearrange(
                    "p (one d) -> p one d", one=1).to_broadcast([128, 8, 64])
                nc.vector.tensor_mul(prod, awin, ob)
            else:
                awin = ar[:, hp * 8:(hp + 1) * 8, :]
                iob = iota64[:, :].rearrange(
                    "p (one d) -> p one d", one=1).to_broadcast([128, 8, 64])
                ofb = offs[hp][:, :].rearrange(
                    "p (k one) -> p k one", one=1).to_broadcast([128, 8, 64])
                mask = pf.tile([128, 8, 64], F32, tag="mask", name="mask")
                nc.vector.tensor_tensor(mask, iob, ofb, op=ALU.is_equal)
                nc.vector.tensor_mul(prod, awin, mask)
            nc.vector.reduce_sum(asel, prod[:, :, :], axis=AX.X)
            nc.vector.tensor_scalar(asel, asel, lw_sb[:, :], None,
                                    op0=ALU.mult)
            nc.vector.tensor_add(lfull[:, s * 8:(s + 1) * 8], asel, contrib)
        if stage <= 4:
            _dbg_out(nc, pred, lfull[:, 0:16], 16)
            return pred
        mn = pf.tile([128, 1], F32, tag="mn", name="mn")
        nc.vector.tensor_reduce(mn, dfull[:, :], axis=AX.X, op=ALU.min)
        e = pf.tile([128, 24], F32, tag="e", name="e")
        nc.scalar.activation(e, dfull[:, :], ACTF.Exp, bias=mn[:, :],
                             scale=-1.0)
        z = pf.tile([128, 1], F32, tag="z", name="z")
        nc.vector.reduce_sum(z, e[:, :], axis=AX.X)
        el = pf.tile([128, 24], F32, tag="el", name="el")
        wl = pf.tile([128, 1], F32, tag="wl", name="wl")
        nc.vector.tensor_mul(el, e, lfull[:, :])
        nc.vector.reduce_sum(wl, el[:, :], axis=AX.X)
        rz = pf.tile([128, 1], F32, tag="rz", name="rz")
        nc.vector.reciprocal(rz, z)
        nc.vector.tensor_mul(R[:, hp:hp + 1], wl, rz)

    if stage <= 5:
        _dbg_out(nc, pred, R[:, :], 2)
        return pred
    ps2 = pp.tile([64, 2], F32, tag="ps", name="ps2")
    nc.tensor.matmul(ps2, oneb_sb[:, :], R[:, :], start=True, stop=True)
    ssum = pf.tile([64, 1], F32, tag="ssum", name="ssum")
    nc.vector.reduce_sum(ssum, ps2[:, :], axis=AX.X)
    outv = pf.tile([64, 1], F32, tag="outv", name="outv")
    nc.scalar.activation(outv, ssum, ACTF.Sigmoid, scale=1.0 / H)
    nc.sync.dma_start(pred[:, :], outv)
    return pred


def build_nc(n_nodes=N_NODES, use_f32r=USE_F32R, pn2_bf16=PN2_BF16, stage=99):
    from contextlib import ExitStack
    nc = bacc.Bacc("TRN2", target_bir_lowering=False, debug=False)
    with tile.TileContext(nc) as tc:
        with ExitStack() as ctx:
            tc._emit_ctx = ctx
            emit(nc, tc, n_nodes, use_f32r, pn2_bf16, stage=stage)
    nc.compile()
    return nc


def pn2_split3(pn2):
    import ml_dtypes
    bf = ml_dtypes.bfloat16
    h = pn2.astype(bf)
    r1 = pn2 - h.astype(np.float32)
    m = r1.astype(bf)
    l = (r1 - m.astype(np.float32)).astype(bf)
    return h, m, l


def host_prep(core, pos, grads, adj, label_w, edges, n_nodes=N_NODES,
              pn2_bf16=PN2_BF16):
    bloc = BLOC
    b0 = core * bloc
    src = edges[0, b0:b0 + bloc].astype(np.int64)
    dst = edges[1, b0:b0 + bloc].astype(np.int64)
    pn2 = np.sum(pos * pos, axis=2)  # (N, H) f32

    im = {}
    for hp in range(2):
        im[f"rhs1_{hp}"] = np.ascontiguousarray(
            np.concatenate([pos[:, 2 * hp, :].T, pos[:, 2 * hp + 1, :].T], 0))
        if pn2_bf16:
            import ml_dtypes
            rows = []
            for hh in (2 * hp, 2 * hp + 1):
                rows.extend(pn2_split3(pn2[:, hh]))
            im[f"rhs2_{hp}"] = np.ascontiguousarray(np.stack(rows, 0)).astype(
                ml_dtypes.bfloat16)
        else:
            im[f"rhs2_{hp}"] = np.ascontiguousarray(
                np.stack([pn2[:, 2 * hp], pn2[:, 2 * hp + 1]], 0))
        im[f"posg_{hp}"] = np.ascontiguousarray(
            np.concatenate([pos[:, 2 * hp, :], pos[:, 2 * hp + 1, :]], 0))
    for s, qi in ((0, src), (1, dst)):
        gi = dst if s == 0 else src
        for hp in range(2):
            L = np.zeros((128, 128), np.float32)
            qt = np.empty((128, D), np.float32)
            gt = np.empty((128, D), np.float32)
            for hi in range(2):
                h = 2 * hp + hi
                Q = pos[qi, h, :]  # (bloc, D)
                L[hi * 64:hi * 64 + D, hi * 64:(hi + 1) * 64] = 2.0 * Q.T
                qt[hi * 64:(hi + 1) * 64] = Q
                gt[hi * 64:(hi + 1) * 64] = grads[gi, h, :]
            im[f"lhsT_{s}{hp}"] = L
            im[f"q_{s}{hp}"] = qt
            im[f"g_{s}{hp}"] = gt
    if pn2_bf16:
        import ml_dtypes
        L2 = np.zeros((6, 128), np.float32)
        L2[0:3, 0:64] = -1.0
        L2[3:6, 64:128] = -1.0
        im["lhsT2"] = L2.astype(ml_dtypes.bfloat16)
    else:
        L2 = np.zeros((2, 128), np.float32)
        L2[0, 0:64] = -1.0
        L2[1, 64:128] = -1.0
        im["lhsT2"] = L2
    im["adjcol"] = np.ascontiguousarray(adj[:, dst])
    im["adjrow"] = np.ascontiguousarray(adj[src, :]).reshape(-1, 64)
    ob = np.zeros((128, 64), np.float32)
    ob[np.arange(128), np.arange(128) % 64] = 1.0
    im["oneb"] = ob
    im["lw"] = np.full((128, 1), float(np.asarray(label_w).reshape(-1)[0]),
                       np.float32)
    return im


_NC_CACHE = {}


def kernel(pos, grads, adj, label_w, edges):
    pos = np.asarray(pos, np.float32)
    grads = np.asarray(grads, np.float32)
    adj = np.asarray(adj, np.float32)
    label_w = np.asarray(label_w, np.float32)
    edges_np = np.asarray(edges)

    key = (N_NODES, USE_F32R, PN2_BF16)
    if key not in _NC_CACHE:
        _NC_CACHE[key] = build_nc(N_NODES, USE_F32R, PN2_BF16)
    nc = _NC_CACHE[key]

    in_maps = [host_prep(r, pos, grads, adj, label_w, edges_np,
                         N_NODES, PN2_BF16) for r in range(N_CORES)]
    res = run_bass_kernel_spmd(nc, in_maps, core_ids=list(range(N_CORES)))
    out = np.concatenate([res.results[r]["pred"][:, 0]
                          for r in range(N_CORES)])
    return out.astype(np.float32)

